# revision 38
# baseline (speedup 1.0000x reference)
"""Trainium2 Bass kernel for MultiHeadSelfAttention (K-only variant), v3.

Math (per batch b):
    K  = x @ Wk.T;  Kh = heads(K)
    S_h = Kh @ Kh.T / sqrt(D);  P_h = softmax(S_h)
    wV_h = P_h @ Kh  (V == K);  out = concat_h(wV) @ Wo.T

Sharding (8 cores): core c handles batch c//2 and query-half c%2 with all
heads.  The query half is selected by rolling x on the host so each core
always computes queries 0:S//2 of its (rolled) sequence; softmax over keys
is order-invariant so rolling the key axis is harmless.

Per-core pipeline (one SPMD NEFF), designed for the serial-PE cost model
(each matmul ~N cycles; tile-position concurrency not assumed):
    xT    = PE-transpose(x) fp32, bf16 on the psum->sbuf copy      [d, s]
    khT   = WkT.T @ xT  (bf16 matmuls, directly in K^T layout)     [e, s]
    kones = XBAR-DMA-transpose(khT) + ones column  [s, (h, hd|1)]
    per (qb, head-pair j):  (heads 2j,2j+1 = partitions 0:64/64:128 of
                             khT chunk ec=j)
      scores strip S_T[k, q] per head (row-tiled pair positions)
      exp: split between ScalarE (true Exp) and VectorE (Schraudolph
        int16 bit-trick, bitcast to bf16); optional symmetric-block
        mirroring via XBAR transposes (qb1 kc<4 = transpose of qb0 kc4-7)
      PV+rowsum fused: [wV_h ; rowsum_h] = [Kh_h | 1].T @ E_h  (M=65)
      recip (DVE, partition 64) -> broadcast via K=1 matmuls at PE row
        64 into [64,512] banks -> normalize wvt8 (DVE TT)
    out = sum_hc wvt8[hc].T @ woT[hc]  (K=64 half-chunks, bf16)
"""

import sys

if "/opt/trn_rl_repo" not in sys.path:
    sys.path.insert(0, "/opt/trn_rl_repo")

import numpy as np

B, S, D = 4, 2048, 512
H = 8
HD = D // H            # 64
P = 128
SH = S // 2            # query half per core = 1024
NCORES = 8
SCALE = 1.0 / np.sqrt(D)
SCHR_A = float(128.0 * SCALE * np.log2(np.e))
SCHR_B = 16250.0

_CACHE = {}


def _build_nc(repeat: int = 1, mode: str = "v3"):
    import concourse.bass as bass  # noqa: F401
    import concourse.tile as tile
    import concourse.mybir as mybir
    from concourse import bacc
    from concourse.masks import make_identity
    from contextlib import ExitStack
    import contextlib

    f32 = mybir.dt.float32
    bf16 = mybir.dt.bfloat16
    i16 = mybir.dt.int16

    nc = bacc.Bacc("TRN2", target_bir_lowering=False, debug=False,
                   num_devices=NCORES)

    x_d = nc.dram_tensor("x", [S, D], f32, kind="ExternalInput").ap()
    wk_d = nc.dram_tensor("Wk", [D, D], f32, kind="ExternalInput").ap()
    wo_d = nc.dram_tensor("Wo", [D, D], f32, kind="ExternalInput").ap()
    out_d = nc.dram_tensor("out", [SH, D], f32, kind="ExternalOutput").ap()

    NSC = S // P           # 16 sequence chunks
    NDC = D // P           # 4 feature chunks
    NQB = SH // 512        # 2 query blocks of 512
    QB = 512
    NHC = 2 * NDC          # 8 e-half-chunks of 64 (== heads)

    # which kc-groups go to the DVE (Schraudolph) vs ScalarE (Exp);
    # spread evenly so neither engine gets a long serial tail
    n_dve = 6
    for tok in mode.split("_"):
        if tok.startswith("s") and tok[1:].isdigit():
            n_dve = int(tok[1:])
    dve_order = [2, 5, 8, 10, 12, 14, 3, 6, 9, 13, 0, 4, 7, 11, 1, 15]
    dve_kcs = set(dve_order[:n_dve])
    sym = "nosym" not in mode
    ebufs = 3 if "e3" in mode else 2
    scbufs = 3

    Exp = mybir.ActivationFunctionType.Exp
    mult = mybir.AluOpType.mult
    add = mybir.AluOpType.add

    with tile.TileContext(nc) as tc:
        loop_cm = tc.For_i(0, repeat, 1) if repeat > 1 else (
            contextlib.nullcontext())
        with loop_cm, ExitStack() as ctx:
            consts = ctx.enter_context(tc.tile_pool(name="consts", bufs=1))
            kpool = ctx.enter_context(tc.tile_pool(name="kpool", bufs=1))
            epool = ctx.enter_context(tc.tile_pool(name="epool", bufs=1))
            vpool = ctx.enter_context(tc.tile_pool(name="vpool", bufs=1))
            opool = ctx.enter_context(tc.tile_pool(name="opool", bufs=1))
            ps = ctx.enter_context(
                tc.tile_pool(name="ps", bufs=1, space="PSUM"))

            ident = consts.tile([P, P], f32)
            make_identity(nc, ident[:])
            ones64 = consts.tile([HD + 1, HD], bf16)
            nc.gpsimd.memset(ones64[:], 1.0)

            wkT = consts.tile([P, NDC, D], bf16)      # [d-part, dc, e]
            woT = consts.tile([HD, NHC, D], bf16)     # [e-half, hc, d']
            khT = kpool.tile([P, NDC, S], bf16)       # K^T [e, s]
            kones = kpool.tile([P, NSC, H, HD + 1], bf16)  # [s,(h,hd|1)]
            nc.gpsimd.memset(kones[:, :, :, HD:HD + 1], 1.0)
            wvt8 = vpool.tile([HD, NHC, SH], bf16)    # wV^T by head

            # table-set preload for Exp (overlaps the front-end DMAs)
            warm = consts.tile([1, 2], bf16)
            warmsrc = consts.tile([1, 2], f32)
            nc.gpsimd.memset(warmsrc[:], 0.0)
            nc.scalar.activation(warm[:], warmsrc[:], Exp, scale=SCALE)

            # ---- phase 0: weights + x transposes + khT + kones ---------
            # psum spans reuse the steady-state "sc" tag (2 banks each)
            with tc.tile_pool(name="stage", bufs=1) as stage:
                wk_sb = stage.tile([P, NDC, D], f32, tag="w", name="wk_sb")
                nc.sync.dma_start(
                    wk_sb[:], wk_d.rearrange("(eo p) d -> p eo d", p=P))
                for dch in range(2):
                    sp = ps.tile([P, 2, 512], f32, tag="sc", bufs=scbufs,
                                 name="wkt_sp")
                    for i in range(2):
                        dc = dch * 2 + i
                        for eo in range(NDC):
                            nc.tensor.transpose(
                                sp[:, i, eo * P:(eo + 1) * P],
                                wk_sb[:, eo, dc * P:(dc + 1) * P],
                                ident[:])
                    nc.vector.tensor_copy(
                        wkT[:, dch * 2:dch * 2 + 2, :], sp[:])

                xT = stage.tile([P, NDC, S], bf16, tag="xT", name="xT")
                for g in range(8):
                    g0 = g * 2
                    x_g = stage.tile([P, 2, 512], f32, tag="x", bufs=2,
                                     name="x_g")
                    nc.sync.dma_start(
                        x_g[:, 0:2, :],
                        x_d[g0 * P:(g0 + 2) * P, :].rearrange(
                            "(two p) d -> p two d", p=P))
                    spt = ps.tile([P, 2, 512], f32, tag="sc", bufs=scbufs,
                                  name="xt_sp")
                    for i in range(2):
                        for dc in range(NDC):
                            nc.tensor.transpose(
                                spt[:, i, dc * P:(dc + 1) * P],
                                x_g[:, i, dc * P:(dc + 1) * P], ident[:])
                    # spt free layout (i, dc, c); xT wants (dc, i, c)
                    src = spt[:].rearrange("p i (dc c) -> p dc i c", dc=4)
                    if g % 2 == 0:
                        nc.vector.tensor_copy(
                            xT[:, :, g0 * P:(g0 + 2) * P].rearrange(
                                "p dc (i c) -> p dc i c", i=2), src)
                    else:
                        nc.scalar.copy(
                            xT[:, :, g0 * P:(g0 + 2) * P].rearrange(
                                "p dc (i c) -> p dc i c", i=2), src)

                wo_sb = stage.tile([P, NDC, D], f32, tag="w", name="wo_sb")
                nc.sync.dma_start(
                    wo_sb[:], wo_d.rearrange("(do p) e -> p do e", p=P))

                for sb in range(4):
                    for ech in range(2):
                        spk = ps.tile([P, 2, 512], f32, tag="sc",
                                      bufs=scbufs, name="kt_sp")
                        for i in range(2):
                            ec = ech * 2 + i
                            for dc in range(NDC):
                                nc.tensor.matmul(
                                    spk[:, i, :],
                                    wkT[:, dc, ec * P:(ec + 1) * P],
                                    xT[:, dc, sb * 512:(sb + 1) * 512],
                                    start=(dc == 0), stop=(dc == NDC - 1))
                        dst = khT[:, ech * 2:ech * 2 + 2,
                                  sb * 512:(sb + 1) * 512]
                        if ech == 0:
                            nc.vector.tensor_copy(dst, spk[:])
                        else:
                            nc.scalar.copy(dst, spk[:])
# (kones is built from k_bf below, after the KT loop)

                # K in [s, e] layout via XBAR (contiguous dst), then a
                # strided copy into kones' gapped [s, (h, hd|1)] layout
                # (XBAR cannot write gapped frees: extra dst dims extend
                # the partition axis, not the free axis)
                k_bf = stage.tile([P, NSC, D], bf16, tag="xT", name="k_bf")
                for sc in range(NSC):
                    for ec in range(NDC):
                        eng = nc.sync if (sc + ec) % 2 == 0 else nc.scalar
                        eng.dma_start_transpose(
                            k_bf[:, sc, ec * P:(ec + 1) * P],
                            khT[:, ec, sc * P:(sc + 1) * P])
                for half in range(8):
                    sl = slice(half * 2, half * 2 + 2)
                    dst = kones[:, sl, :, 0:HD]
                    src = k_bf[:, sl, :].rearrange(
                        "p sc (h e) -> p sc h e", h=H)
                    if half % 2 == 0:
                        nc.vector.tensor_copy(dst, src)
                    else:
                        nc.scalar.copy(dst, src)

                # Wo transposed into 64-partition half-chunks [e64, d']
                for hch in range(4):
                    spw = ps.tile([HD, 2, 512], f32, tag="sc", bufs=scbufs,
                                  name="wot_sp")
                    for i in range(2):
                        hc = hch * 2 + i
                        for do in range(NDC):
                            nc.tensor.transpose(
                                spw[:, i, do * P:(do + 1) * P],
                                wo_sb[:, do, hc * HD:(hc + 1) * HD],
                                ident[:])
                    nc.vector.tensor_copy(
                        woT[:, hch * 2:hch * 2 + 2, :], spw[:])

            # ---- steady state: (qb, head-pair) periods -----------------
            def emit_scores(j, qb, e_t, hooks, skip=()):
                for kc in range(NSC):
                    if kc in skip:
                        if kc in hooks:
                            hooks[kc]()
                        continue
                    sp = ps.tile([P, 2, 512], f32, tag="sc", bufs=scbufs,
                                 name="sc")
                    for hi in range(2):
                        nc.tensor.matmul(
                            sp[:, hi, :],
                            khT[hi * HD:(hi + 1) * HD, j,
                                kc * P:(kc + 1) * P],
                            khT[hi * HD:(hi + 1) * HD, j,
                                qb * QB:(qb + 1) * QB],
                            start=True, stop=True,
                            tile_position=(hi * HD, 0))
                    dst = e_t[:, kc, :, :]
                    if kc in dve_kcs:
                        with nc.allow_low_precision(
                                reason="schraudolph exp"):
                            nc.vector.tensor_scalar(
                                dst.bitcast(i16), sp[:],
                                SCHR_A, SCHR_B, mult, add)
                    else:
                        nc.scalar.activation(dst, sp[:], Exp, scale=SCALE)
                    if kc in hooks:
                        hooks[kc]()

            def esl(e_t, mirror, kc, hi):
                # exp tile source: mirrored transpose for qb1 kc<4
                if mirror is not None and kc < 4:
                    return mirror[:, kc, hi, :]
                return e_t[:, kc, hi, :]

            def emit_pv(j, qb, e_t, mirror=None):
                # fused PV+rowsum per head: lhsT = [Kh_h | 1] (M=65);
                # row 64 of the psum bank is the rowsum
                pv_a = ps.tile([HD + 1, 512], f32, tag="pv", bufs=2,
                               name="pv_a")
                pv_b = ps.tile([HD + 1, 512], f32, tag="pv", bufs=2,
                               name="pv_b")
                for kc in range(NSC):
                    nc.tensor.matmul(
                        pv_a[:, :], kones[:, kc, 2 * j, :],
                        esl(e_t, mirror, kc, 0),
                        start=(kc == 0), stop=(kc == NSC - 1))
                for kc in range(NSC):
                    nc.tensor.matmul(
                        pv_b[:, :], kones[:, kc, 2 * j + 1, :],
                        esl(e_t, mirror, kc, 1),
                        start=(kc == 0), stop=(kc == NSC - 1))
                return pv_a, pv_b

            def emit_norm(j, qb, pv):
                pv_a, pv_b = pv
                recip = vpool.tile([HD + 1, 2, 512], bf16, tag="recip",
                                   bufs=2, name="recip")
                with nc.allow_low_precision(reason="bf16 recip"):
                    nc.vector.reciprocal(recip[HD:HD + 1, 0, :],
                                         pv_a[HD:HD + 1, :])
                    nc.vector.reciprocal(recip[HD:HD + 1, 1, :],
                                         pv_b[HD:HD + 1, :])
                # copy wV rows out before the rb allocs reuse the banks
                nc.vector.tensor_copy(
                    wvt8[:, 2 * j, qb * QB:(qb + 1) * QB], pv_a[0:HD, :])
                nc.vector.tensor_copy(
                    wvt8[:, 2 * j + 1, qb * QB:(qb + 1) * QB],
                    pv_b[0:HD, :])
                # partition-broadcast each head's recip row via K=1
                # matmuls at PE row 64 into [64,512] banks (pv tag reuse)
                rb_a = ps.tile([HD, 512], f32, tag="pv", bufs=2,
                               name="rb_a")
                rb_b = ps.tile([HD, 512], f32, tag="pv", bufs=2,
                               name="rb_b")
                nc.tensor.matmul(rb_a[:, :], ones64[HD:HD + 1, :],
                                 recip[HD:HD + 1, 0, :],
                                 start=True, stop=True,
                                 tile_position=(HD, 0))
                nc.tensor.matmul(rb_b[:, :], ones64[HD:HD + 1, :],
                                 recip[HD:HD + 1, 1, :],
                                 start=True, stop=True,
                                 tile_position=(HD, 0))
                nc.vector.tensor_tensor(
                    wvt8[:, 2 * j, qb * QB:(qb + 1) * QB],
                    wvt8[:, 2 * j, qb * QB:(qb + 1) * QB],
                    rb_a[:, :], mult)
                nc.vector.tensor_tensor(
                    wvt8[:, 2 * j + 1, qb * QB:(qb + 1) * QB],
                    wvt8[:, 2 * j + 1, qb * QB:(qb + 1) * QB],
                    rb_b[:, :], mult)

            def emit_outproj(qb, half):
                # half in {0,1}: query chunks qb*4 + 2*half + {0,1}
                qc0 = qb * 4 + 2 * half
                po = ps.tile([P, 2, 512], f32, tag="sc", bufs=scbufs,
                             name="po")
                for j2 in range(2):
                    qc = qc0 + j2
                    for hc in range(NHC):
                        nc.tensor.matmul(
                            po[:, j2, :],
                            wvt8[:, hc, qc * P:(qc + 1) * P],
                            woT[:, hc, :],
                            start=(hc == 0), stop=(hc == NHC - 1))
                o_sb = opool.tile([P, 2, 512], f32, tag="osb", bufs=2,
                                  name="o_sb")
                nc.vector.tensor_copy(o_sb[:], po[:])
                nc.sync.dma_start(
                    out_d[qc0 * P:(qc0 + 2) * P, :].rearrange(
                        "(two p) d -> p two d", p=P),
                    o_sb[:])

            dbg = "dbg" in mode
            if dbg:
                d_kht = nc.dram_tensor(
                    "d_kht", [P, NDC, S], bf16, kind="ExternalOutput").ap()
                d_kones = nc.dram_tensor(
                    "d_kones", [P, NSC, H, HD + 1], bf16,
                    kind="ExternalOutput").ap()
                d_et = nc.dram_tensor(
                    "d_et", [P, NSC, 2, 512], bf16,
                    kind="ExternalOutput").ap()
                d_wvt = nc.dram_tensor(
                    "d_wvt", [HD, NHC, SH], bf16,
                    kind="ExternalOutput").ap()
                nc.sync.dma_start(d_kht, khT[:])
                nc.sync.dma_start(d_kones, kones[:])

            pending = None
            outq = []
            mirrors = {}
            for qb in range(NQB):
                for j in range(4):
                    e_t = epool.tile([P, NSC, 2, 512], bf16, tag="E",
                                     bufs=ebufs, name="e_t")
                    mirror = mirrors.get(j) if (sym and qb == 1) else None
                    hooks = {}
                    if pending is not None:
                        pj, pqb, pe, pm = pending
                        state = {}

                        def h3(pj=pj, pqb=pqb, pe=pe, pm=pm, state=state):
                            state["pv"] = emit_pv(pj, pqb, pe, pm)

                        def h8(pj=pj, pqb=pqb, state=state):
                            emit_norm(pj, pqb, state["pv"])
                            if pj == 3:
                                outq.extend([(pqb, 0), (pqb, 1)])

                        def h11():
                            if outq:
                                emit_outproj(*outq.pop(0))

                        def h13():
                            if outq:
                                emit_outproj(*outq.pop(0))

                        hooks = {3: h3, 8: h8, 11: h11, 13: h13}
                    emit_scores(j, qb, e_t, hooks,
                                skip=(range(4) if mirror is not None
                                      else ()))
                    if dbg and j == 0 and qb == 0:
                        nc.sync.dma_start(d_et, e_t[:])
                    if sym and qb == 0:
                        # mirror the symmetric block: E(qb1)[kc<4] is the
                        # transpose of E(qb0)[kc 4..7] (XBAR, off-engine)
                        m_t = epool.tile([P, 4, 2, 512], bf16, tag="M",
                                         bufs=4, name="m_t")
                        for m in range(4):
                            for hi in range(2):
                                nc.sync.dma_start_transpose(
                                    m_t[:, 0:4, hi, m * P:(m + 1) * P],
                                    e_t[:, 4 + m, hi, :])
                        mirrors[j] = m_t
                    pending = (j, qb, e_t, mirror)

            pj, pqb, pe, pm = pending
            pv = emit_pv(pj, pqb, pe, pm)
            emit_norm(pj, pqb, pv)
            outq.extend([(pqb, 0), (pqb, 1)])
            while outq:
                emit_outproj(*outq.pop(0))
            if dbg:
                nc.sync.dma_start(d_wvt, wvt8[:])

    nc.compile()
    return nc


def _get_nc(repeat: int = 1, mode: str = "v3"):
    key = ("nc", repeat, mode)
    if key not in _CACHE:
        _CACHE[key] = _build_nc(repeat, mode)
    return _CACHE[key]


def _shard_inputs(x, Wk, Wo):
    in_maps = []
    for c in range(NCORES):
        b, half = c // 2, c % 2
        xb = x[b]
        if half:
            xb = np.roll(xb, -SH, axis=0)
        in_maps.append({"x": np.ascontiguousarray(xb), "Wk": Wk, "Wo": Wo})
    return in_maps


def kernel(x: np.ndarray, Wk: np.ndarray, Wo: np.ndarray, _trace=False):
    from concourse import bass_utils

    nc = _get_nc()
    x = np.asarray(x, dtype=np.float32)
    Wk = np.ascontiguousarray(np.asarray(Wk, dtype=np.float32))
    Wo = np.ascontiguousarray(np.asarray(Wo, dtype=np.float32))

    in_maps = _shard_inputs(x, Wk, Wo)

    res = bass_utils.run_bass_kernel_spmd(
        nc, in_maps, core_ids=list(range(NCORES)), trace=_trace)

    out = np.empty((B, S, D), dtype=np.float32)
    for c in range(NCORES):
        b, half = c // 2, c % 2
        out[b, half * SH:(half + 1) * SH] = res.results[c]["out"]
    if _trace:
        _CACHE["last_results"] = res
    return out


# revision 40
# speedup vs baseline: 1.1332x; 1.1332x over previous
"""Trainium2 Bass kernel for MultiHeadSelfAttention (K-only variant), v3.

Math (per batch b):
    K  = x @ Wk.T;  Kh = heads(K)
    S_h = Kh @ Kh.T / sqrt(D);  P_h = softmax(S_h)
    wV_h = P_h @ Kh  (V == K);  out = concat_h(wV) @ Wo.T

Sharding (8 cores): core c handles batch c//2 and query-half c%2 with all
heads.  The query half is selected by rolling x on the host so each core
always computes queries 0:S//2 of its (rolled) sequence; softmax over keys
is order-invariant so rolling the key axis is harmless.

Per-core pipeline (one SPMD NEFF), designed for the serial-PE cost model
(each matmul ~N cycles; tile-position concurrency not assumed):
    xT    = PE-transpose(x) fp32, bf16 on the psum->sbuf copy      [d, s]
    khT   = WkT.T @ xT  (bf16 matmuls, directly in K^T layout)     [e, s]
    kones = XBAR-DMA-transpose(khT) + ones column  [s, (h, hd|1)]
    per (qb, head-pair j):  (heads 2j,2j+1 = partitions 0:64/64:128 of
                             khT chunk ec=j)
      scores strip S_T[k, q] per head (row-tiled pair positions)
      exp: split between ScalarE (true Exp) and VectorE (Schraudolph
        int16 bit-trick, bitcast to bf16); optional symmetric-block
        mirroring via XBAR transposes (qb1 kc<4 = transpose of qb0 kc4-7)
      PV+rowsum fused: [wV_h ; rowsum_h] = [Kh_h | 1].T @ E_h  (M=65)
      recip (DVE, partition 64) -> broadcast via K=1 matmuls at PE row
        64 into [64,512] banks -> normalize wvt8 (DVE TT)
    out = sum_hc wvt8[hc].T @ woT[hc]  (K=64 half-chunks, bf16)
"""

import sys

if "/opt/trn_rl_repo" not in sys.path:
    sys.path.insert(0, "/opt/trn_rl_repo")

import numpy as np

B, S, D = 4, 2048, 512
H = 8
HD = D // H            # 64
P = 128
SH = S // 2            # query half per core = 1024
NCORES = 8
SCALE = 1.0 / np.sqrt(D)
SCHR_A = float(128.0 * SCALE * np.log2(np.e))
SCHR_B = 16250.0

_CACHE = {}


def _build_nc(repeat: int = 1, mode: str = "v3"):
    import concourse.bass as bass  # noqa: F401
    import concourse.tile as tile
    import concourse.mybir as mybir
    from concourse import bacc
    from concourse.masks import make_identity
    from contextlib import ExitStack
    import contextlib

    f32 = mybir.dt.float32
    bf16 = mybir.dt.bfloat16
    i16 = mybir.dt.int16

    nc = bacc.Bacc("TRN2", target_bir_lowering=False, debug=False,
                   num_devices=NCORES)

    x_d = nc.dram_tensor("x", [S, D], f32, kind="ExternalInput").ap()
    wk_d = nc.dram_tensor("Wk", [D, D], f32, kind="ExternalInput").ap()
    wo_d = nc.dram_tensor("Wo", [D, D], f32, kind="ExternalInput").ap()
    out_d = nc.dram_tensor("out", [SH, D], f32, kind="ExternalOutput").ap()

    NSC = S // P           # 16 sequence chunks
    NDC = D // P           # 4 feature chunks
    NQB = SH // 512        # 2 query blocks of 512
    QB = 512
    NHC = 2 * NDC          # 8 e-half-chunks of 64 (== heads)

    # which kc-groups go to the DVE (Schraudolph) vs ScalarE (Exp);
    # spread evenly so neither engine gets a long serial tail
    n_dve = 6
    for tok in mode.split("_"):
        if tok.startswith("s") and tok[1:].isdigit():
            n_dve = int(tok[1:])
    dve_order = [2, 5, 8, 10, 12, 14, 3, 6, 9, 13, 0, 4, 7, 11, 1, 15]
    dve_kcs = set(dve_order[:n_dve])
    sym = "nosym" not in mode
    ebufs = 3 if "e3" in mode else 2
    scbufs = 3

    Exp = mybir.ActivationFunctionType.Exp
    mult = mybir.AluOpType.mult
    add = mybir.AluOpType.add

    with tile.TileContext(nc) as tc:
        loop_cm = tc.For_i(0, repeat, 1) if repeat > 1 else (
            contextlib.nullcontext())
        with loop_cm, ExitStack() as ctx:
            consts = ctx.enter_context(tc.tile_pool(name="consts", bufs=1))
            kpool = ctx.enter_context(tc.tile_pool(name="kpool", bufs=1))
            epool = ctx.enter_context(tc.tile_pool(name="epool", bufs=1))
            vpool = ctx.enter_context(tc.tile_pool(name="vpool", bufs=1))
            opool = ctx.enter_context(tc.tile_pool(name="opool", bufs=1))
            ps = ctx.enter_context(
                tc.tile_pool(name="ps", bufs=1, space="PSUM"))

            ident = consts.tile([P, P], f32)
            make_identity(nc, ident[:])
            ones64 = consts.tile([HD + 1, HD], bf16)
            nc.gpsimd.memset(ones64[:], 1.0)

            wkT = consts.tile([P, NDC, D], bf16)      # [d-part, dc, e]
            woT = consts.tile([HD, NHC, D], bf16)     # [e-half, hc, d']
            khT = kpool.tile([P, NDC, S], bf16)       # K^T [e, s]
            kones = kpool.tile([P, NSC, H, HD + 1], bf16)  # [s,(h,hd|1)]
            nc.gpsimd.memset(kones[:, :, :, HD:HD + 1], 1.0)
            wvt8 = vpool.tile([HD, NHC, SH], bf16)    # wV^T by head

            # table-set preload for Exp (overlaps the front-end DMAs)
            warm = consts.tile([1, 2], bf16)
            warmsrc = consts.tile([1, 2], f32)
            nc.gpsimd.memset(warmsrc[:], 0.0)
            nc.scalar.activation(warm[:], warmsrc[:], Exp, scale=SCALE)

            # ---- phase 0: weights + x transposes + khT + kones ---------
            # psum spans reuse the steady-state "sc" tag (2 banks each)
            with tc.tile_pool(name="stage", bufs=1) as stage:
                wk_sb = stage.tile([P, NDC, D], f32, tag="w", name="wk_sb")
                nc.sync.dma_start(
                    wk_sb[:], wk_d.rearrange("(eo p) d -> p eo d", p=P))
                for dch in range(2):
                    sp = ps.tile([P, 2, 512], f32, tag="sc", bufs=scbufs,
                                 name="wkt_sp")
                    for i in range(2):
                        dc = dch * 2 + i
                        for eo in range(NDC):
                            nc.tensor.transpose(
                                sp[:, i, eo * P:(eo + 1) * P],
                                wk_sb[:, eo, dc * P:(dc + 1) * P],
                                ident[:])
                    nc.vector.tensor_copy(
                        wkT[:, dch * 2:dch * 2 + 2, :], sp[:])

                xT = stage.tile([P, NDC, S], bf16, tag="xT", name="xT")
                for g in range(8):
                    g0 = g * 2
                    x_g = stage.tile([P, 2, 512], f32, tag="x", bufs=2,
                                     name="x_g")
                    nc.sync.dma_start(
                        x_g[:, 0:2, :],
                        x_d[g0 * P:(g0 + 2) * P, :].rearrange(
                            "(two p) d -> p two d", p=P))
                    spt = ps.tile([P, 2, 512], f32, tag="sc", bufs=scbufs,
                                  name="xt_sp")
                    for i in range(2):
                        for dc in range(NDC):
                            nc.tensor.transpose(
                                spt[:, i, dc * P:(dc + 1) * P],
                                x_g[:, i, dc * P:(dc + 1) * P], ident[:])
                    # spt free layout (i, dc, c); xT wants (dc, i, c)
                    src = spt[:].rearrange("p i (dc c) -> p dc i c", dc=4)
                    if g % 2 == 0:
                        nc.vector.tensor_copy(
                            xT[:, :, g0 * P:(g0 + 2) * P].rearrange(
                                "p dc (i c) -> p dc i c", i=2), src)
                    else:
                        nc.scalar.copy(
                            xT[:, :, g0 * P:(g0 + 2) * P].rearrange(
                                "p dc (i c) -> p dc i c", i=2), src)

                wo_sb = stage.tile([P, NDC, D], f32, tag="w", name="wo_sb")
                nc.sync.dma_start(
                    wo_sb[:], wo_d.rearrange("(do p) e -> p do e", p=P))

                for sb in range(4):
                    for ech in range(2):
                        spk = ps.tile([P, 2, 512], f32, tag="sc",
                                      bufs=scbufs, name="kt_sp")
                        for i in range(2):
                            ec = ech * 2 + i
                            for dc in range(NDC):
                                nc.tensor.matmul(
                                    spk[:, i, :],
                                    wkT[:, dc, ec * P:(ec + 1) * P],
                                    xT[:, dc, sb * 512:(sb + 1) * 512],
                                    start=(dc == 0), stop=(dc == NDC - 1))
                        dst = khT[:, ech * 2:ech * 2 + 2,
                                  sb * 512:(sb + 1) * 512]
                        if ech == 0:
                            nc.vector.tensor_copy(dst, spk[:])
                        else:
                            nc.scalar.copy(dst, spk[:])
# (kones is built from k_bf below, after the KT loop)

                # K in [s, e] layout via XBAR (contiguous dst), then a
                # strided copy into kones' gapped [s, (h, hd|1)] layout
                # (XBAR cannot write gapped frees: extra dst dims extend
                # the partition axis, not the free axis)
                k_bf = stage.tile([P, NSC, D], bf16, tag="xT", name="k_bf")
                for sb in range(4):
                    for ec in range(NDC):
                        eng = nc.sync if ec % 2 == 0 else nc.scalar
                        eng.dma_start_transpose(
                            k_bf[:, 4 * sb:4 * sb + 4, ec * P:(ec + 1) * P],
                            khT[:, ec, sb * 512:(sb + 1) * 512])
                for half in range(8):
                    sl = slice(half * 2, half * 2 + 2)
                    dst = kones[:, sl, :, 0:HD]
                    src = k_bf[:, sl, :].rearrange(
                        "p sc (h e) -> p sc h e", h=H)
                    if half % 2 == 0:
                        nc.vector.tensor_copy(dst, src)
                    else:
                        nc.scalar.copy(dst, src)

                # Wo transposed into 64-partition half-chunks [e64, d']
                for hch in range(4):
                    spw = ps.tile([HD, 2, 512], f32, tag="sc", bufs=scbufs,
                                  name="wot_sp")
                    for i in range(2):
                        hc = hch * 2 + i
                        for do in range(NDC):
                            nc.tensor.transpose(
                                spw[:, i, do * P:(do + 1) * P],
                                wo_sb[:, do, hc * HD:(hc + 1) * HD],
                                ident[:])
                    nc.vector.tensor_copy(
                        woT[:, hch * 2:hch * 2 + 2, :], spw[:])

            # ---- steady state: (qb, head-pair) periods -----------------
            def emit_scores(j, qb, e_t, hooks, skip=()):
                for kc in range(NSC):
                    if kc in skip:
                        if kc in hooks:
                            hooks[kc]()
                        continue
                    sp = ps.tile([P, 2, 512], f32, tag="sc", bufs=scbufs,
                                 name="sc")
                    for hi in range(2):
                        nc.tensor.matmul(
                            sp[:, hi, :],
                            khT[hi * HD:(hi + 1) * HD, j,
                                kc * P:(kc + 1) * P],
                            khT[hi * HD:(hi + 1) * HD, j,
                                qb * QB:(qb + 1) * QB],
                            start=True, stop=True,
                            tile_position=(hi * HD, 0))
                    dst = e_t[:, kc, :, :]
                    if kc in dve_kcs:
                        with nc.allow_low_precision(
                                reason="schraudolph exp"):
                            nc.vector.tensor_scalar(
                                dst.bitcast(i16), sp[:],
                                SCHR_A, SCHR_B, mult, add)
                    else:
                        nc.scalar.activation(dst, sp[:], Exp, scale=SCALE)
                    if kc in hooks:
                        hooks[kc]()

            def esl(e_t, mirror, kc, hi):
                # exp tile source: mirrored transpose for qb1 kc<4
                if mirror is not None and kc < 4:
                    return mirror[:, kc, hi, :]
                return e_t[:, kc, hi, :]

            def emit_pv(j, qb, e_t, mirror=None):
                # fused PV+rowsum per head: lhsT = [Kh_h | 1] (M=65);
                # row 64 of the psum bank is the rowsum
                pv_a = ps.tile([HD + 1, 512], f32, tag="pv", bufs=2,
                               name="pv_a")
                pv_b = ps.tile([HD + 1, 512], f32, tag="pv", bufs=2,
                               name="pv_b")
                for kc in range(NSC):
                    nc.tensor.matmul(
                        pv_a[:, :], kones[:, kc, 2 * j, :],
                        esl(e_t, mirror, kc, 0),
                        start=(kc == 0), stop=(kc == NSC - 1))
                for kc in range(NSC):
                    nc.tensor.matmul(
                        pv_b[:, :], kones[:, kc, 2 * j + 1, :],
                        esl(e_t, mirror, kc, 1),
                        start=(kc == 0), stop=(kc == NSC - 1))
                return pv_a, pv_b

            def emit_norm(j, qb, pv):
                pv_a, pv_b = pv
                recip = vpool.tile([HD + 1, 2, 512], bf16, tag="recip",
                                   bufs=2, name="recip")
                with nc.allow_low_precision(reason="bf16 recip"):
                    nc.vector.reciprocal(recip[HD:HD + 1, 0, :],
                                         pv_a[HD:HD + 1, :])
                    nc.vector.reciprocal(recip[HD:HD + 1, 1, :],
                                         pv_b[HD:HD + 1, :])
                # copy wV rows out before the rb allocs reuse the banks
                nc.vector.tensor_copy(
                    wvt8[:, 2 * j, qb * QB:(qb + 1) * QB], pv_a[0:HD, :])
                nc.vector.tensor_copy(
                    wvt8[:, 2 * j + 1, qb * QB:(qb + 1) * QB],
                    pv_b[0:HD, :])
                # partition-broadcast each head's recip row via K=1
                # matmuls at PE row 64 into [64,512] banks (pv tag reuse)
                rb_a = ps.tile([HD, 512], f32, tag="pv", bufs=2,
                               name="rb_a")
                rb_b = ps.tile([HD, 512], f32, tag="pv", bufs=2,
                               name="rb_b")
                nc.tensor.matmul(rb_a[:, :], ones64[HD:HD + 1, :],
                                 recip[HD:HD + 1, 0, :],
                                 start=True, stop=True,
                                 tile_position=(HD, 0))
                nc.tensor.matmul(rb_b[:, :], ones64[HD:HD + 1, :],
                                 recip[HD:HD + 1, 1, :],
                                 start=True, stop=True,
                                 tile_position=(HD, 0))
                nc.vector.tensor_tensor(
                    wvt8[:, 2 * j, qb * QB:(qb + 1) * QB],
                    wvt8[:, 2 * j, qb * QB:(qb + 1) * QB],
                    rb_a[:, :], mult)
                nc.vector.tensor_tensor(
                    wvt8[:, 2 * j + 1, qb * QB:(qb + 1) * QB],
                    wvt8[:, 2 * j + 1, qb * QB:(qb + 1) * QB],
                    rb_b[:, :], mult)

            def emit_outproj(qb, half):
                # half in {0,1}: query chunks qb*4 + 2*half + {0,1}
                qc0 = qb * 4 + 2 * half
                po = ps.tile([P, 2, 512], f32, tag="sc", bufs=scbufs,
                             name="po")
                for j2 in range(2):
                    qc = qc0 + j2
                    for hc in range(NHC):
                        nc.tensor.matmul(
                            po[:, j2, :],
                            wvt8[:, hc, qc * P:(qc + 1) * P],
                            woT[:, hc, :],
                            start=(hc == 0), stop=(hc == NHC - 1))
                o_sb = opool.tile([P, 2, 512], f32, tag="osb", bufs=2,
                                  name="o_sb")
                nc.vector.tensor_copy(o_sb[:], po[:])
                nc.sync.dma_start(
                    out_d[qc0 * P:(qc0 + 2) * P, :].rearrange(
                        "(two p) d -> p two d", p=P),
                    o_sb[:])

            dbg = "dbg" in mode
            if dbg:
                d_kht = nc.dram_tensor(
                    "d_kht", [P, NDC, S], bf16, kind="ExternalOutput").ap()
                d_kones = nc.dram_tensor(
                    "d_kones", [P, NSC, H, HD + 1], bf16,
                    kind="ExternalOutput").ap()
                d_et = nc.dram_tensor(
                    "d_et", [P, NSC, 2, 512], bf16,
                    kind="ExternalOutput").ap()
                d_wvt = nc.dram_tensor(
                    "d_wvt", [HD, NHC, SH], bf16,
                    kind="ExternalOutput").ap()
                nc.sync.dma_start(d_kht, khT[:])
                nc.sync.dma_start(d_kones, kones[:])

            pending = None
            outq = []
            mirrors = {}
            for qb in range(NQB):
                for j in range(4):
                    e_t = epool.tile([P, NSC, 2, 512], bf16, tag="E",
                                     bufs=ebufs, name="e_t")
                    mirror = mirrors.get(j) if (sym and qb == 1) else None
                    hooks = {}
                    if pending is not None:
                        pj, pqb, pe, pm = pending
                        state = {}

                        def h3(pj=pj, pqb=pqb, pe=pe, pm=pm, state=state):
                            state["pv"] = emit_pv(pj, pqb, pe, pm)

                        def h8(pj=pj, pqb=pqb, state=state):
                            emit_norm(pj, pqb, state["pv"])
                            if pj == 3:
                                outq.extend([(pqb, 0), (pqb, 1)])

                        def h11():
                            if outq:
                                emit_outproj(*outq.pop(0))

                        def h13():
                            if outq:
                                emit_outproj(*outq.pop(0))

                        hooks = {3: h3, 8: h8, 11: h11, 13: h13}
                    emit_scores(j, qb, e_t, hooks,
                                skip=(range(4) if mirror is not None
                                      else ()))
                    if dbg and j == 0 and qb == 0:
                        nc.sync.dma_start(d_et, e_t[:])
                    if sym and qb == 0:
                        # mirror the symmetric block: E(qb1)[kc<4] is the
                        # transpose of E(qb0)[kc 4..7] (XBAR, off-engine)
                        m_t = epool.tile([P, 4, 2, 512], bf16, tag="M",
                                         bufs=4, name="m_t")
                        for m in range(4):
                            for hi in range(2):
                                nc.sync.dma_start_transpose(
                                    m_t[:, 0:4, hi, m * P:(m + 1) * P],
                                    e_t[:, 4 + m, hi, :])
                        mirrors[j] = m_t
                    pending = (j, qb, e_t, mirror)

            pj, pqb, pe, pm = pending
            pv = emit_pv(pj, pqb, pe, pm)
            emit_norm(pj, pqb, pv)
            outq.extend([(pqb, 0), (pqb, 1)])
            while outq:
                emit_outproj(*outq.pop(0))
            if dbg:
                nc.sync.dma_start(d_wvt, wvt8[:])

    nc.compile()
    return nc


def _get_nc(repeat: int = 1, mode: str = "v3"):
    key = ("nc", repeat, mode)
    if key not in _CACHE:
        _CACHE[key] = _build_nc(repeat, mode)
    return _CACHE[key]


def _shard_inputs(x, Wk, Wo):
    in_maps = []
    for c in range(NCORES):
        b, half = c // 2, c % 2
        xb = x[b]
        if half:
            xb = np.roll(xb, -SH, axis=0)
        in_maps.append({"x": np.ascontiguousarray(xb), "Wk": Wk, "Wo": Wo})
    return in_maps


def kernel(x: np.ndarray, Wk: np.ndarray, Wo: np.ndarray, _trace=False):
    from concourse import bass_utils

    nc = _get_nc()
    x = np.asarray(x, dtype=np.float32)
    Wk = np.ascontiguousarray(np.asarray(Wk, dtype=np.float32))
    Wo = np.ascontiguousarray(np.asarray(Wo, dtype=np.float32))

    in_maps = _shard_inputs(x, Wk, Wo)

    res = bass_utils.run_bass_kernel_spmd(
        nc, in_maps, core_ids=list(range(NCORES)), trace=_trace)

    out = np.empty((B, S, D), dtype=np.float32)
    for c in range(NCORES):
        b, half = c // 2, c % 2
        out[b, half * SH:(half + 1) * SH] = res.results[c]["out"]
    if _trace:
        _CACHE["last_results"] = res
    return out
